# revision 1
# baseline (speedup 1.0000x reference)
"""Trainium2 Bass kernel for MultiHeadAttention with shape-bias penalty (v3).

Data-parallel over batch (16 batches -> 2 per core on 8 cores).
v3 highlights:
  - all heavy matmuls in bf16 (fp32 costs 4 cyc/row on PE, bf16 costs 1);
    q/k/v converted to bf16 on the Pool engine before PE transposes.
  - penalty folded into the logits PSUM accumulation via (-I) @ pen.
  - dist^2 as one K=5 fp32 matmul with a positivity bias row (no clamp).
  - softmax-normalize via a single tensor_scalar divide (no reciprocal),
    rsqrt fusion for LayerNorm / embedding normalization.
  - weights staged through chunk DMAs on the software-DGE queue so the
    first batch's input loads aren't stuck behind 9.4MB of weights.
  - double-buffered qhT/khT/vh/pen/qin so batch b+1's projections overlap
    batch b's attention.
"""

import json

import numpy as np

import concourse.bass as bass
import concourse.mybir as mybir
import concourse.tile as tile
from concourse.bass_utils import run_bass_kernel_spmd
from concourse.masks import make_identity
from concourse.vector_clock import ScopedClock

# ---------------------------------------------------------------------------
# Problem constants (hardcoded per spec)
# ---------------------------------------------------------------------------
B = 16
N_CORES = 8
B_LOC = B // N_CORES  # batches per core
N = 577  # sequence length (1 CLS + 576 patches)
D = 768  # model dim
H = 12  # heads
DK = 64  # head dim
TEMP = float(DK) ** 0.5
LN_EPS = 1e-6
EPS = 1e-12
DELTA = 1e-3  # dist^2 positivity bias (>> fp32 cancellation noise ~2e-4)
P = 128  # partitions

NCH = (N + P - 1) // P  # 5 token chunks: 128,128,128,128,65
CH_SZ = [min(P, N - c * P) for c in range(NCH)]
NDC = D // P  # 6 d-chunks
FP = mybir.dt.float32
BF = mybir.dt.bfloat16

MAX_WAITS_PER_INST = 1


# ---------------------------------------------------------------------------
# Walrus compatibility: this toolchain accepts at most one sem wait per
# instruction; Tile can attach several. Split extras onto injected NoOps
# (same engine, in-order) in the serialized BIR, and chain the final drain.
# ---------------------------------------------------------------------------
def _split_multiwait_bir(bir_bytes: bytes) -> bytes:
    bir = json.loads(bir_bytes)
    counter = [0]
    for f in bir["functions"]:
        for blk in f["blocks"]:
            new = []
            for inst in blk["instructions"]:
                si = inst.get("sync_info")
                if si:
                    waits = si.get("on_wait") or []
                    if len(waits) > MAX_WAITS_PER_INST:
                        for w in waits[:-MAX_WAITS_PER_INST]:
                            counter[0] += 1
                            new.append(
                                {
                                    "debug": inst.get("debug", 0),
                                    "engine": inst["engine"],
                                    "ins": [],
                                    "name": f"I-wsplit-{counter[0]}",
                                    "opcode": "NoOp",
                                    "outs": [],
                                    "sync_info": {"on_update": [], "on_wait": [w]},
                                }
                            )
                        si["on_wait"] = waits[-MAX_WAITS_PER_INST:]
                    ups = si.get("on_update") or []
                    if len(ups) > 1 and inst["opcode"] not in (
                        "DMACopy",
                        "DMATranspose",
                        "TriggeredCopy",
                    ):
                        for u in ups[1:]:
                            counter[0] += 1
                            new.append(inst)
                            inst = {
                                "debug": inst.get("debug", 0),
                                "engine": inst["engine"],
                                "ins": [],
                                "name": f"I-usplit-{counter[0]}",
                                "opcode": "NoOp",
                                "outs": [],
                                "sync_info": {"on_update": [u], "on_wait": []},
                            }
                        si["on_update"] = [ups[0]]
                new.append(inst)
            blk["instructions"] = new
    return json.dumps(bir).encode()


class _SplitDrainTileContext(tile.TileContext):
    def _drain_and_barrier(self, tick_clock, wait_clock):
        import bass_rust as _bass_rust

        drain_inst = self.nc.sync.drain()
        wait_clock.add_sem_waits(
            drain_inst.ins, ScopedClock({None: tick_clock.global_clock})
        )
        waits = list(drain_inst.ins.sync_info.on_wait)
        if len(waits) > MAX_WAITS_PER_INST:
            si = drain_inst.ins.sync_info
            si.on_wait = waits[:MAX_WAITS_PER_INST]
            drain_inst.ins.sync_info = si
            for i in range(MAX_WAITS_PER_INST, len(waits), MAX_WAITS_PER_INST):
                extra = self.nc.sync.drain()
                extra.ins.sync_info = _bass_rust.SyncInfo(
                    on_wait=waits[i : i + MAX_WAITS_PER_INST], on_update=[]
                )
        self.nc.all_engine_barrier()
        assert self.sems is not None
        popped = self.nc._tile_sem_poison_stack.pop()
        assert popped is self._sem_poison
        self.nc.clear_and_free_semaphores(list(self.sems.allocated().values()))
        self.nc.all_engine_barrier()


def _install_bir_postpass(nc):
    orig = nc.to_json_bytes
    nc.to_json_bytes = lambda: _split_multiwait_bir(orig())
    return nc


# ---------------------------------------------------------------------------
# Kernel body
# ---------------------------------------------------------------------------
def _ts(c):
    return slice(c * P, c * P + CH_SZ[c])


def build_kernel(n_batches=B_LOC, reps=1):
    AF = mybir.ActivationFunctionType
    ALU = mybir.AluOpType

    nc = bass.Bass("TRN2", target_bir_lowering=False)
    dq = nc.dram_tensor("q", [n_batches, N, D], FP, kind="ExternalInput")
    dk = nc.dram_tensor("k", [n_batches, N, D], FP, kind="ExternalInput")
    dv = nc.dram_tensor("v", [n_batches, N, D], FP, kind="ExternalInput")
    dpos = nc.dram_tensor("pos", [n_batches, N - 1, 2], FP, kind="ExternalInput")
    demb = nc.dram_tensor("emb", [n_batches, N, D], FP, kind="ExternalInput")
    dwq = nc.dram_tensor("w_qs", [D, D], FP, kind="ExternalInput")
    dwk = nc.dram_tensor("w_ks", [D, D], FP, kind="ExternalInput")
    dwv = nc.dram_tensor("w_vs", [D, D], FP, kind="ExternalInput")
    dwf = nc.dram_tensor("w_fc", [D, D], FP, kind="ExternalInput")
    dgam = nc.dram_tensor("gamma", [D], FP, kind="ExternalInput")
    dbet = nc.dram_tensor("beta", [D], FP, kind="ExternalInput")
    dout = nc.dram_tensor("out", [n_batches, N, D], FP, kind="ExternalOutput")

    with _SplitDrainTileContext(nc) as tc:
        if reps > 1:
            with tc.For_i(0, reps, 1):
                _kernel_body(
                    tc, nc, AF, ALU, n_batches,
                    dq, dk, dv, dpos, demb, dwq, dwk, dwv, dwf, dgam, dbet, dout,
                )
        else:
            _kernel_body(
                tc, nc, AF, ALU, n_batches,
                dq, dk, dv, dpos, demb, dwq, dwk, dwv, dwf, dgam, dbet, dout,
            )
    _install_bir_postpass(nc)
    return nc


def _kernel_body(
    tc, nc, AF, ALU, n_batches,
    dq, dk, dv, dpos, demb, dwq, dwk, dwv, dwf, dgam, dbet, dout,
):
    from contextlib import ExitStack

    assert n_batches == B_LOC == 2

    with ExitStack() as ctx:
        singles = ctx.enter_context(tc.tile_pool(name="singles", bufs=1))
        wbf = ctx.enter_context(tc.tile_pool(name="wbf", bufs=1))
        xinp = ctx.enter_context(tc.tile_pool(name="xinp", bufs=1))
        xc = ctx.enter_context(tc.tile_pool(name="xc", bufs=6))
        xcb = ctx.enter_context(tc.tile_pool(name="xcb", bufs=7))
        qrb = ctx.enter_context(tc.tile_pool(name="qrb", bufs=10))
        xT = ctx.enter_context(tc.tile_pool(name="xT", bufs=2))
        oTp = ctx.enter_context(tc.tile_pool(name="oTp", bufs=2))
        hT = ctx.enter_context(tc.tile_pool(name="hT", bufs=2))
        vpool = ctx.enter_context(tc.tile_pool(name="vp", bufs=2))
        penp = ctx.enter_context(tc.tile_pool(name="pen", bufs=1))
        epool = ctx.enter_context(tc.tile_pool(name="E", bufs=2))
        scr = ctx.enter_context(tc.tile_pool(name="scr", bufs=2))
        small = ctx.enter_context(tc.tile_pool(name="small", bufs=4))
        outp = ctx.enter_context(tc.tile_pool(name="outp", bufs=2))
        # PSUM: 8 banks total (4 tags x 2 bufs)
        psb = ctx.enter_context(tc.tile_pool(name="psb", bufs=2, space="PSUM"))
        pa = ctx.enter_context(tc.tile_pool(name="pa", bufs=2, space="PSUM"))
        ptl = ctx.enter_context(tc.tile_pool(name="ptl", bufs=2, space="PSUM"))
        po = ctx.enter_context(tc.tile_pool(name="po", bufs=2, space="PSUM"))

        # ---- constants ----
        identF = singles.tile([P, P], FP)
        make_identity(nc, identF)
        identB = singles.tile([P, P], BF)
        nc.vector.tensor_copy(identB, identF)
        eps12 = singles.tile([P, 1], FP)
        nc.vector.memset(eps12, EPS)
        epsln = singles.tile([P, 1], FP)
        nc.vector.memset(epsln, LN_EPS)
        gam_b = singles.tile([P, D], FP)
        nc.sync.dma_start(out=gam_b, in_=dgam[None, :].to_broadcast((P, D)))
        bet_b = singles.tile([P, D], FP)
        nc.sync.dma_start(out=bet_b, in_=dbet[None, :].to_broadcast((P, D)))

        dve_copy = nc.vector.tensor_copy
        pool_copy = nc.gpsimd.tensor_copy

        def act_copy(out, in_):
            nc.scalar.activation(out=out, in_=in_, func=AF.Copy)

        def load_weight(dram, tag, scale=None, conv=None):
            wt = wbf.tile([P, NDC, D], BF, tag=tag)
            for dc in range(NDC):
                st = xc.tile([P, D], FP, tag="xc")
                nc.sync.dma_start(out=st, in_=dram[dc * P : (dc + 1) * P, :])
                if scale is not None:
                    nc.vector.tensor_scalar_mul(wt[:, dc, :], st, scale)
                else:
                    conv(wt[:, dc, :], st)
            return wt

        def t_chunk(src, dst, c, eng):
            """6 PE transposes of one [sz, D] bf16 chunk + one PSUM copy."""
            sz = CH_SZ[c]
            ps = psb.tile([P, NDC, P], BF, tag="psb")
            for dc in range(NDC):
                nc.tensor.transpose(
                    ps[0:P, dc, 0:sz],
                    src[0:sz, dc * P : (dc + 1) * P],
                    identB[0:sz, 0:sz],
                )
            eng(dst[:, :, _ts(c)], ps[:, :, 0:sz])

        def load_chunks(dram, b):
            chunks = []
            for c in range(NCH):
                sz = CH_SZ[c]
                t = xc.tile([P, D], FP, tag="xc")
                nc.scalar.dma_start(out=t[0:sz, :], in_=dram[b, _ts(c), :])
                chunks.append(t)
            return chunks

        def to_bf(chunks_or_slices, pool=None, tag="xcb"):
            out = []
            for sl in chunks_or_slices:
                t = (pool or xcb).tile([P, D], BF, tag=tag)
                pool_copy(t, sl)
                out.append(t)
            return out

        def proj_oc(w, xT_t, dst, oc, eng):
            """One output-chunk (128 rows) of a [o, n] projection."""
            for i0, isz in [(0, 512), (512, 65)]:
                psm = pa.tile([P, 512], FP, tag="pa")
                for dc in range(NDC):
                    nc.tensor.matmul(
                        psm[0:P, 0:isz],
                        w[:, dc, oc * P : (oc + 1) * P],
                        xT_t[:, dc, i0 : i0 + isz],
                        start=(dc == 0),
                        stop=(dc == NDC - 1),
                    )
                eng(dst[:, oc, i0 : i0 + isz], psm[0:P, 0:isz])

        def vh_chunk(xvT_t, wv_t, vh_t, c, eng):
            sz = CH_SZ[c]
            for og in range(2):
                psm = pa.tile([P, 512], FP, tag="pa")
                for dc in range(NDC):
                    nc.tensor.matmul(
                        psm[0:sz, 0:384],
                        xvT_t[:, dc, _ts(c)],
                        wv_t[:, dc, og * 384 : (og + 1) * 384],
                        start=(dc == 0),
                        stop=(dc == NDC - 1),
                    )
                eng(
                    vh_t[0:sz, c, og * 6 : (og + 1) * 6, 0:DK],
                    psm[0:sz, 0:384].rearrange("p (h e) -> p h e", h=6),
                )

        st = [dict() for _ in range(2)]  # per-batch tiles

        def e_pos_pen_gen(b):
            """emb normalize + transpose, positions, penalty, exp(-pen)."""
            ptile = xinp.tile([P, NCH, 5], FP, tag="ptile")
            nc.gpsimd.memset(ptile[:, :, 0:3], 0.0)
            nc.gpsimd.memset(ptile[:, :, 3:5], 1.0)
            nc.sync.dma_start(out=ptile[1:P, 0, 0:2], in_=dpos[b, 0 : P - 1, :])
            nc.sync.dma_start(
                out=ptile[:, 1 : NCH - 1, 0:2],
                in_=dpos[b, P - 1 : (NCH - 1) * P - 1, :].rearrange(
                    "(c p) d -> p c d", p=P
                ),
            )
            nc.sync.dma_start(
                out=ptile[0 : CH_SZ[NCH - 1], NCH - 1, 0:2],
                in_=dpos[b, (NCH - 1) * P - 1 : N - 1, :],
            )
            for c in range(NCH):
                sz = CH_SZ[c]
                sq2 = small.tile([P, 2], FP, tag="sq2")
                nc.scalar.activation(
                    out=sq2[0:sz],
                    in_=ptile[0:sz, c, 0:2],
                    func=AF.Square,
                    accum_out=ptile[0:sz, c, 2:3],
                )
            ptileB = xinp.tile([P, NCH, 5], FP, tag="ptileB")
            nc.gpsimd.memset(ptileB[:, :, 2:3], 1.0)
            nc.gpsimd.memset(ptileB[:, :, 4:5], DELTA)
            nc.gpsimd.tensor_scalar_mul(
                ptileB[:, :, 0:2], ptile[:, :, 0:2], -2.0
            )
            pool_copy(ptileB[:, :, 3:4], ptile[:, :, 2:3])
            yield
            A5 = xinp.tile([5, N], FP, tag="A5")
            B5 = xinp.tile([5, N], FP, tag="B5")
            for c in range(NCH):
                sz = CH_SZ[c]
                psA = pa.tile([P, 512], FP, tag="pa")
                nc.tensor.transpose(
                    psA[0:5, 0:sz], ptile[0:sz, c, :], identF[0:sz, 0:sz]
                )
                nc.tensor.transpose(
                    psA[0:5, 128 : 128 + sz], ptileB[0:sz, c, :],
                    identF[0:sz, 0:sz],
                )
                dve_copy(A5[0:5, _ts(c)], psA[0:5, 0:sz])
                dve_copy(B5[0:5, _ts(c)], psA[0:5, 128 : 128 + sz])
            # precompute all upper-triangle dist blocks (no eT dependency):
            # for batch 0 this runs inside the opening DMA wait
            dists = scr.tile([P, 15, P], BF, tag="dists", bufs=1)
            di = 0
            for c in range(NCH):
                sz = CH_SZ[c]
                for ic in range(c, NCH):
                    szi = CH_SZ[ic]
                    psd = pa.tile([P, 512], FP, tag="pa")
                    nc.tensor.matmul(
                        psd[0:sz, 0:szi],
                        A5[0:5, _ts(c)],
                        B5[0:5, ic * P : ic * P + szi],
                        start=True,
                        stop=True,
                    )
                    nc.scalar.activation(
                        out=dists[0:sz, di, 0:szi], in_=psd[0:sz, 0:szi],
                        func=AF.Sqrt,
                    )
                    di += 1
                yield

            yield
            ech = load_chunks(demb, b)
            ebf_chunks = []
            for c in range(NCH):
                sz = CH_SZ[c]
                nsum = small.tile([P, 1], FP, tag="nsum")
                ebf = xcb.tile([P, D], BF, tag="xcb")
                nc.scalar.activation(
                    out=ebf[0:sz, :],
                    in_=ech[c][0:sz, :],
                    func=AF.Square,
                    accum_out=nsum[0:sz],
                )
                nc.scalar.activation(
                    out=nsum[0:sz], in_=nsum[0:sz], func=AF.Sqrt, bias=eps12[0:sz]
                )
                rin = small.tile([P, 1], FP, tag="rin")
                nc.vector.reciprocal(rin[0:sz], nsum[0:sz])
                nc.gpsimd.tensor_scalar_mul(
                    ebf[0:sz, :], ech[c][0:sz, :], rin[0:sz]
                )
                ebf_chunks.append(ebf)
                yield
            nc.gpsimd.memset(ebf_chunks[0][0:1, :], 0.0)  # zero CLS row
            eT = xT.tile([P, NDC, N], BF, tag="xT")
            for c in range(NCH):
                t_chunk(ebf_chunks[c], eT, c, [act_copy, dve_copy][c % 2])
                yield

            # symmetric penalty: upper-triangle blocks, transpose the rest
            pen = penp.tile([P, NCH, N], BF, tag="pen")
            di = 0
            for c in range(NCH):
                sz = CH_SZ[c]
                for ic in range(c, NCH):
                    szi = CH_SZ[ic]
                    i0 = ic * P
                    dist = dists[:, di, :]
                    di += 1
                    pss = pa.tile([P, 512], FP, tag="pa")
                    for dc in range(NDC):
                        nc.tensor.matmul(
                            pss[0:sz, 0:szi],
                            eT[:, dc, _ts(c)],
                            eT[:, dc, i0 : i0 + szi],
                            start=(dc == 0),
                            stop=(dc == NDC - 1),
                        )
                    nc.vector.tensor_tensor(
                        out=pen[0:sz, c, i0 : i0 + szi],
                        in0=pss[0:sz, 0:szi],
                        in1=dist[0:sz, 0:szi],
                        op=ALU.mult,
                    )
                    if ic > c:
                        pstp = psb.tile([P, NDC, P], BF, tag="psb")
                        nc.tensor.transpose(
                            pstp[0:szi, 0, 0:sz],
                            pen[0:sz, c, i0 : i0 + szi],
                            identB[0:sz, 0:sz],
                        )
                        dve_copy(
                            pen[0:szi, ic, c * P : c * P + sz],
                            pstp[0:szi, 0, 0:sz],
                        )
                    yield

            enp = penp.tile([P, NCH, N], BF, tag="enp")
            st[b]["enp"] = enp
            for c in range(NCH):
                nc.scalar.activation(
                    out=enp[:, c, :], in_=pen[:, c, :], func=AF.Exp, scale=-1.0
                )
                yield

        # ================= attention emitters =================
        def emit_s_exp(b, h):
            qhT, khT, enp = st[b]["qhT"], st[b]["khT"], st[b]["enp"]
            oc, orow = h // 2, (h % 2) * DK
            et = epool.tile([P, NCH, N], BF, tag="E")
            ptail = ptl.tile([P, NCH, 65], FP, tag="ptl")
            for c in range(NCH):
                sz = CH_SZ[c]
                pss = pa.tile([P, 512], FP, tag="pa")
                nc.tensor.matmul(
                    pss[0:sz, 0:512],
                    khT[orow : orow + DK, oc, _ts(c)],
                    qhT[orow : orow + DK, oc, 0:512],
                    start=True,
                    stop=True,
                )
                nc.scalar.activation(
                    out=et[0:sz, c, 0:512], in_=pss[0:sz, 0:512], func=AF.Exp
                )
                nc.vector.tensor_tensor(
                    out=et[0:sz, c, 0:512],
                    in0=et[0:sz, c, 0:512],
                    in1=enp[0:sz, c, 0:512],
                    op=ALU.mult,
                )
                nc.tensor.matmul(
                    ptail[0:sz, c, :],
                    khT[orow : orow + DK, oc, _ts(c)],
                    qhT[orow : orow + DK, oc, 512:577],
                    start=True,
                    stop=True,
                )
            nc.scalar.activation(
                out=et[:, 0:4, 512:577], in_=ptail[:, 0:4, :], func=AF.Exp
            )
            nc.scalar.activation(
                out=et[0 : CH_SZ[4], 4, 512:577],
                in_=ptail[0 : CH_SZ[4], 4, :],
                func=AF.Exp,
            )
            nc.gpsimd.tensor_tensor(
                out=et[:, 0:4, 512:577],
                in0=et[:, 0:4, 512:577],
                in1=enp[:, 0:4, 512:577],
                op=ALU.mult,
            )
            nc.gpsimd.tensor_tensor(
                out=et[0 : CH_SZ[4], 4, 512:577],
                in0=et[0 : CH_SZ[4], 4, 512:577],
                in1=enp[0 : CH_SZ[4], 4, 512:577],
                op=ALU.mult,
            )
            return et

        def emit_ev(b, h, et):
            vh, Ot = st[b]["vh"], st[b]["Ot"]
            for ic in range(NCH):
                szi = CH_SZ[ic]
                pso = po.tile([P, DK + 1], FP, tag="po")
                for c in range(NCH):
                    sz = CH_SZ[c]
                    nc.tensor.matmul(
                        pso[0:szi, :],
                        et[0:sz, c, ic * P : ic * P + szi],
                        vh[0:sz, c, h, :],
                        start=(c == 0),
                        stop=(c == NCH - 1),
                    )
                rcp = small.tile([P, 1], FP, tag="rcp")
                nc.vector.reciprocal(rcp[0:szi], pso[0:szi, DK : DK + 1])
                nc.vector.tensor_scalar_mul(
                    Ot[0:szi, ic, h * DK : (h + 1) * DK],
                    pso[0:szi, 0:DK],
                    rcp[0:szi],
                )

        def ot_transpose(b):
            Ot = st[b]["Ot"]
            oT = oTp.tile([P, NDC, N], BF, tag="oT")
            for c in range(NCH):
                t_chunk(Ot[:, c, :], oT, c, dve_copy)
            st[b]["oT"] = oT

        def outproj_chunk(b, c):
            oT, qbf = st[b]["oT"], st[b]["qbf"]
            sz = CH_SZ[c]
            xo = outp.tile([P, D], FP, tag="xo")
            ssum = small.tile([P, 2], FP, tag="ssum")
            for og in range(2):
                psf = pa.tile([P, 512], FP, tag="pa")
                for dc in range(NDC):
                    nc.tensor.matmul(
                        psf[0:sz, 0:384],
                        oT[:, dc, _ts(c)],
                        wf[:, dc, og * 384 : (og + 1) * 384],
                        start=(dc == 0),
                        stop=False,
                    )
                # residual as a 7th accumulation term: I.T @ q_chunk
                nc.tensor.matmul(
                    psf[0:sz, 0:384],
                    identB[0:sz, 0:sz],
                    qbf[c][0:sz, og * 384 : (og + 1) * 384],
                    start=False,
                    stop=True,
                )
                # copy to SBUF; the row-sum falls out of the accumulator
                nc.scalar.activation(
                    out=xo[0:sz, og * 384 : (og + 1) * 384],
                    in_=psf[0:sz, 0:384],
                    func=AF.Copy,
                    accum_out=ssum[0:sz, og : og + 1],
                )
            # second moment via one Square pass (output is scratch)
            sq = xcb.tile([P, D], BF, tag="xcb")
            q2 = small.tile([P, 1], FP, tag="q2")
            nc.scalar.activation(
                out=sq[0:sz, :],
                in_=xo[0:sz, :],
                func=AF.Square,
                accum_out=q2[0:sz],
            )
            # mu = (s0+s1)/D ; var = q2/D - mu^2 ; rstd = 1/sqrt(var+eps)
            mu = small.tile([P, 1], FP, tag="mu")
            nc.vector.tensor_tensor(
                out=mu[0:sz], in0=ssum[0:sz, 0:1], in1=ssum[0:sz, 1:2],
                op=ALU.add,
            )
            nc.vector.tensor_scalar_mul(mu[0:sz], mu[0:sz], 1.0 / D)
            mu2 = small.tile([P, 1], FP, tag="mu2")
            nc.vector.tensor_tensor(
                out=mu2[0:sz], in0=mu[0:sz], in1=mu[0:sz], op=ALU.mult
            )
            var = small.tile([P, 1], FP, tag="var")
            nc.vector.scalar_tensor_tensor(
                out=var[0:sz], in0=q2[0:sz], scalar=1.0 / D, in1=mu2[0:sz],
                op0=ALU.mult, op1=ALU.subtract,
            )
            rstd = small.tile([P, 1], FP, tag="rstd")
            nc.scalar.activation(
                out=rstd[0:sz], in_=var[0:sz], func=AF.Sqrt, bias=epsln[0:sz]
            )
            nc.vector.reciprocal(rstd[0:sz], rstd[0:sz])
            nc.vector.scalar_tensor_tensor(
                out=xo[0:sz],
                in0=xo[0:sz],
                scalar=mu[0:sz],
                in1=gam_b[0:sz],
                op0=ALU.subtract,
                op1=ALU.mult,
            )
            yt = outp.tile([P, D], FP, tag="yt")
            nc.vector.scalar_tensor_tensor(
                out=yt[0:sz],
                in0=xo[0:sz],
                scalar=rstd[0:sz],
                in1=bet_b[0:sz],
                op0=ALU.mult,
                op1=ALU.add,
            )
            nc.sync.dma_start(out=dout[b, _ts(c), :], in_=yt[0:sz])

        # ================= batch-0 prep (inline) =================
        g0 = e_pos_pen_gen(0)
        drain_n = lambda g, k: [next(g, None) for _ in range(k)]
        # issue the q loads first so their DMA issues aren't queued behind
        # the dist-precompute's sqrt ops on the depth-0 ACT sequencer
        qch0 = load_chunks(dq, 0)
        drain_n(g0, 1)  # positions + A5/B5 + dist blocks fill the DMA wait
        qbf0 = to_bf([t[0:P, :] for t in qch0], pool=qrb, tag="qrb")
        st[0]["qbf"] = qbf0
        wq = load_weight(dwq, "wq", scale=1.0 / TEMP)

        xqT0 = xT.tile([P, NDC, N], BF, tag="xT")
        for c in range(NCH):
            t_chunk(qbf0[c], xqT0, c, dve_copy)
        qhT0 = hT.tile([P, NDC, N], BF, tag="hTq")
        for oc in range(NDC):
            proj_oc(wq, xqT0, qhT0, oc, dve_copy)
        st[0]["qhT"] = qhT0

        kch0 = load_chunks(dk, 0)
        wk = load_weight(dwk, "wk", conv=dve_copy)
        kbf0 = to_bf([t[0:P, :] for t in kch0])
        xkT0 = xT.tile([P, NDC, N], BF, tag="xT")
        for c in range(NCH):
            t_chunk(kbf0[c], xkT0, c, dve_copy)
        khT0 = hT.tile([P, NDC, N], BF, tag="hTk")
        for oc in range(NDC):
            proj_oc(wk, xkT0, khT0, oc, dve_copy)
        st[0]["khT"] = khT0

        wv = load_weight(dwv, "wv", conv=pool_copy)
        wf = load_weight(dwf, "wf", conv=dve_copy)

        vch0 = load_chunks(dv, 0)
        vbf0 = to_bf([t[0:P, :] for t in vch0])
        xvT0 = xT.tile([P, NDC, N], BF, tag="xT")
        for c in range(NCH):
            t_chunk(vbf0[c], xvT0, c, dve_copy)
        vh0 = vpool.tile([P, NCH, H, DK + 1], BF, tag="vh")
        nc.gpsimd.memset(vh0[:, :, :, DK : DK + 1], 1.0)
        for c in range(NCH):
            vh_chunk(xvT0, wv, vh0, c, dve_copy)
        st[0]["vh"] = vh0

        for _ in g0:
            pass
        ot0 = epool.tile([P, NCH, D], BF, tag="Ot", bufs=1)
        st[0]["Ot"] = ot0

        # ============ batch-1 q/k/v prep as interleavable units ============
        def b1_qkv_gen():
            qch = load_chunks(dq, 1)
            qbf = to_bf([t[0:P, :] for t in qch], pool=qrb, tag="qrb")
            st[1]["qbf"] = qbf
            yield
            xqT = xT.tile([P, NDC, N], BF, tag="xT")
            for c in range(NCH):
                t_chunk(qbf[c], xqT, c, dve_copy)
                yield
            qhT = hT.tile([P, NDC, N], BF, tag="hTq")
            st[1]["qhT"] = qhT
            for oc in range(NDC):
                proj_oc(wq, xqT, qhT, oc, dve_copy)
                yield
            kch = load_chunks(dk, 1)
            kbf = to_bf([t[0:P, :] for t in kch])
            yield
            xkT = xT.tile([P, NDC, N], BF, tag="xT")
            for c in range(NCH):
                t_chunk(kbf[c], xkT, c, dve_copy)
                yield
            khT = hT.tile([P, NDC, N], BF, tag="hTk")
            st[1]["khT"] = khT
            for oc in range(NDC):
                proj_oc(wk, xkT, khT, oc, dve_copy)
                yield
            vch = load_chunks(dv, 1)
            vbf = to_bf([t[0:P, :] for t in vch])
            yield
            xvT = xT.tile([P, NDC, N], BF, tag="xT")
            for c in range(NCH):
                t_chunk(vbf[c], xvT, c, dve_copy)
                yield
            vh1 = vpool.tile([P, NCH, H, DK + 1], BF, tag="vh")
            nc.gpsimd.memset(vh1[:, :, :, DK : DK + 1], 1.0)
            st[1]["vh"] = vh1
            for c in range(NCH):
                vh_chunk(xvT, wv, vh1, c, dve_copy)
                yield

        # ============ batch-0 attention + batch-1 prep interleaved =========
        g1 = b1_qkv_gen()

        def drain(g, k):
            for _ in range(k):
                try:
                    next(g)
                except StopIteration:
                    return False
            return True

        prev = None
        for h in range(H):
            et = emit_s_exp(0, h)
            if prev is not None:
                emit_ev(0, h - 1, prev)
            prev = et
            drain(g1, 4)
        emit_ev(0, H - 1, prev)
        drain(g1, 200)
        ot_transpose(0)

        # batch-1 penalty etc. (between the two attention phases)
        for _ in e_pos_pen_gen(1):
            pass
        ot1 = epool.tile([P, NCH, D], BF, tag="Ot", bufs=1)
        st[1]["Ot"] = ot1

        # ======= batch-1 attention + batch-0 output proj interleaved =======
        op0 = list(range(NCH))
        prev = None
        for h in range(H):
            et = emit_s_exp(1, h)
            if prev is not None:
                emit_ev(1, h - 1, prev)
            prev = et
            if False and op0:
                outproj_chunk(0, op0.pop(0))
        emit_ev(1, H - 1, prev)
        ot_transpose(1)
        # alternate the two batches' output projections so the LN chains of
        # one batch overlap the matmuls of the other
        for c in range(NCH):
            outproj_chunk(0, c)
            outproj_chunk(1, c)


# ---------------------------------------------------------------------------
# Host-side entry point
# ---------------------------------------------------------------------------
_NC_CACHE = {}


def _get_nc():
    if "nc" not in _NC_CACHE:
        _NC_CACHE["nc"] = build_kernel(B_LOC)
    return _NC_CACHE["nc"]


def _make_in_maps(q, k, v, patch_positions, patch_embeddings,
                  w_qs, w_ks, w_vs, w_fc, ln_gamma, ln_beta):
    f32 = lambda x: np.ascontiguousarray(np.asarray(x), dtype=np.float32)
    shared = {
        "w_qs": f32(w_qs), "w_ks": f32(w_ks), "w_vs": f32(w_vs),
        "w_fc": f32(w_fc), "gamma": f32(ln_gamma), "beta": f32(ln_beta),
    }
    in_maps = []
    for c in range(N_CORES):
        sl = slice(c * B_LOC, (c + 1) * B_LOC)
        in_maps.append(
            {
                "q": f32(q[sl]),
                "k": f32(k[sl]),
                "v": f32(v[sl]),
                "pos": f32(patch_positions[sl]),
                "emb": f32(patch_embeddings[sl]),
                **shared,
            }
        )
    return in_maps


def kernel(q, k, v, patch_positions, patch_embeddings,
           w_qs, w_ks, w_vs, w_fc, ln_gamma, ln_beta):
    nc = _get_nc()
    in_maps = _make_in_maps(q, k, v, patch_positions, patch_embeddings,
                            w_qs, w_ks, w_vs, w_fc, ln_gamma, ln_beta)
    res = run_bass_kernel_spmd(nc, in_maps, core_ids=list(range(N_CORES)))
    return np.concatenate([r["out"] for r in res.results], axis=0)

